# revision 65
# baseline (speedup 1.0000x reference)
"""Trainium2 Bass kernel for COTREC-style GNN message passing.

Math (reference):
    cur1 = S @ emb                      (S sparse [N,N], 1M nnz)
    cur2 = S @ cur1
    item = (emb + cur1 + cur2) / 3
    sess = meanpool_sessions(item)      ([B, E])
    ... small dense tail (DA @ ..., w_sess, l2norm) ...

Device decomposition (8 cores, SPMD single program, per-core data via inputs):
  * The embedding table is uploaded bf16, row-sharded (one shard per core),
    and AllGathered on device into a full DRAM copy per core - 8x less
    host->device traffic than replicating it.
  * Row-shard nodes: core m owns rows [m*NLOC, (m+1)*NLOC).
  * cur1 shard computed locally as a stream of 128-slot tiles: each tile
    gathers up to 128 edge-source rows (indirect DMA, OOB pad slots
    skipped via bounds_check) and one mask-matmul [128 slots x 128 rows]
    segment-sums them into the 128-row group's PSUM accumulator. Masks are
    built ON DEVICE from per-slot (val bf16, row-offset u8) via a
    broadcast is_equal against an iota - only 7B/slot goes over the wire.
  * cur2 never materialized. The session pooling is pushed through the
    graph: sess*3*len = P01@emb + P01@cur1 + (P01@S)@cur1, where P01/Q01
    structures are host-computed integer index work, processed as the same
    kind of tile stream. Per-core partial pooled sums [E, B] -> one small
    AllReduce.
  * Dense tail replicated on all cores (DA = D @ A precomputed on host,
    shipped bf16); core 0's output returned.
  * All per-core arrays ship in 4 consolidated [128, x] blobs (bf16 /
    int32 / uint8 / f32) to minimise per-buffer dispatch overhead.
"""

import os
import sys
from contextlib import ExitStack

import numpy as np
import ml_dtypes

BF16NP = ml_dtypes.bfloat16

for _p in ("/opt/trn_rl_repo", os.path.expanduser("~/.axon_site/_ro/trn_rl_repo")):
    if os.path.isdir(_p) and _p not in sys.path:
        sys.path.append(_p)

import concourse.bacc as bacc
import concourse.bass as bass
import concourse.tile as tile
from concourse import mybir
from concourse.masks import make_identity

F32 = mybir.dt.float32
BF16 = mybir.dt.bfloat16
I32 = mybir.dt.int32
I8 = mybir.dt.int8


class Cfg:
    def __init__(self, N=100000, NNZ=1000000, B=512, L=50, M=8, EMB=112):
        self.N, self.NNZ, self.B, self.L, self.M = N, NNZ, B, L, M
        self.EMB = EMB
        # shard rows per core: multiple of 128, cover N
        self.NLOC = ((N + M - 1) // M + 127) // 128 * 128
        self.NTOT = self.NLOC * M          # rows in allgathered table
        self.G = self.NLOC // 128          # 128-row groups per core
        self.NWS = B // 128                # session windows (128 wide)
        self.BT = B // 128                 # tail b-tiles


# ---------------------------------------------------------------------------
# Host preprocessing: pure integer/layout work + permutation of input floats.
# ---------------------------------------------------------------------------

def _csr_expand(rowptr, rows):
    """For each r in rows return concatenated [rowptr[r], rowptr[r+1]) ranges."""
    deg = rowptr[rows + 1] - rowptr[rows]
    total = int(deg.sum())
    if total == 0:
        return np.zeros(0, np.int64), deg
    cum = np.cumsum(deg)
    out = np.arange(total, dtype=np.int64) - np.repeat(cum - deg, deg) \
        + np.repeat(rowptr[rows], deg)
    return out, deg


def _tile_stream(cfg, per_core, n_win, oob, uniform=False):
    """Pack per-core (window, col-offset, gather-idx, val) edge lists into a
    shared-layout stream of 128-slot tiles.

    per_core[m] = (win, off, gidx, val) arrays.  Tile capacity per window is
    the max over cores (the SPMD program must be identical on every core).
    With uniform=True every window gets the same capacity (hardware-loop
    friendly: tile t belongs to window t // cap).
    Returns (idx[M,128,T], val[M,128,T], off[M,128,T], wmap[T], T).
    """
    c = cfg
    cnts = np.zeros((c.M, n_win), np.int64)
    for m, (w, _, _, _) in enumerate(per_core):
        cnts[m] = np.bincount(w, minlength=n_win)
    caps = np.maximum(1, (cnts.max(axis=0) + 127) // 128)    # tiles per window
    if uniform:
        caps[:] = caps.max()
    tbase = np.zeros(n_win, np.int64)
    tbase[1:] = np.cumsum(caps)[:-1]
    T = int(caps.sum())
    wmap = np.zeros(T, np.int64)
    for wi in range(n_win):
        wmap[tbase[wi]:tbase[wi] + caps[wi]] = wi
    idx = np.full((c.M, 128, T), oob, np.int32)
    val = np.zeros((c.M, 128, T), BF16NP)
    off = np.full((c.M, 128, T), -1, np.int8)            # -1 = empty slot
    for m, (w, o, gidx, vv) in enumerate(per_core):
        so = np.argsort(w, kind="stable")
        w, o, gidx, vv = w[so], o[so], gidx[so], vv[so]
        starts = np.zeros(n_win, np.int64)
        starts[1:] = np.cumsum(cnts[m])[:-1]
        k = np.arange(len(w)) - starts[w]               # rank within window
        t = tbase[w] + k // 128
        p = k % 128
        idx[m, p, t] = gidx
        val[m, p, t] = vv.astype(BF16NP)
        off[m, p, t] = o
    return idx, val, off, wmap, T


def prep(cfg, inputs):
    """Build per-core input arrays + the (core-independent) program plan."""
    c = cfg
    emb = np.asarray(inputs["embedding"], np.float32)
    av = np.asarray(inputs["adj_vals"], np.float32)
    ar = np.asarray(inputs["adj_rows"], np.int64)
    ac = np.asarray(inputs["adj_cols"], np.int64)
    D = np.asarray(inputs["D"], np.float32)
    A = np.asarray(inputs["A"], np.float32)
    si = np.asarray(inputs["session_item"], np.int64)
    sl = np.asarray(inputs["session_len"], np.float32)
    w_sess = np.asarray(inputs["w_sess"], np.float32)

    # int8 per-row-quantized sharded embedding table (zero rows pad to NTOT);
    # the per-row scale is folded into the edge-mask values (bf16) below.
    s_row = np.maximum(np.abs(emb).max(axis=1) / 127.0, 1e-12)  # [N]
    emb_q = np.zeros((c.NTOT, c.EMB), np.int8)
    emb_q[:c.N] = np.clip(np.round(emb / s_row[:, None]), -127, 127)

    # session refs: (b, col) for non-pad items
    b_ref = np.repeat(np.arange(c.B, dtype=np.int64), c.L)
    it_ref = si.ravel()
    keep = it_ref > 0
    b_ref, col_ref = b_ref[keep], it_ref[keep] - 1       # cols in [0, N)

    # CSR of S by row (for Q01 = P01 @ S)
    order = np.argsort(ar, kind="stable")
    ar_s, ac_s, av_s = ar[order], ac[order], av[order]
    rowptr = np.searchsorted(ar_s, np.arange(c.N + 1)).astype(np.int64)
    epos, deg = _csr_expand(rowptr, col_ref)
    q_b = np.repeat(b_ref, deg)
    q_c = ac_s[epos]
    q_v = av_s[epos]

    # prune L1 edges to rows actually referenced by T1/T2
    ref_mask = np.zeros(c.N, bool)
    ref_mask[col_ref] = True
    ref_mask[q_c] = True
    ekeep = ref_mask[ar]
    er, ec, ev = ar[ekeep], ac[ekeep], av[ekeep]

    own_e = er // c.NLOC
    own_ref = col_ref // c.NLOC
    own_q = q_c // c.NLOC

    # ---- L1 stream: windows are the 128-row groups of the local shard ----
    l1_pc = []
    for m in range(c.M):
        sel = own_e == m
        r = er[sel] - m * c.NLOC
        l1_pc.append((r // 128, r % 128, ec[sel], ev[sel] * s_row[ec[sel]]))
    l1_idx, l1_val, l1_off, _, l1T = _tile_stream(c, l1_pc, c.G, c.N,
                                                  uniform=True)
    l1_cap = l1T // c.G

    # ---- sess stream A: P01 @ emb (gathers emb at col_ref) ----------------
    # ---- sess stream B: (P01 + Q01) @ cur1 (gathers local cur1 rows) ------
    sa_pc = []
    sb_pc = []
    for m in range(c.M):
        selr = own_ref == m
        sa_pc.append((b_ref[selr] // 128, b_ref[selr] % 128, col_ref[selr],
                      s_row[col_ref[selr]]))
        selq = own_q == m
        bb = np.concatenate([b_ref[selr], q_b[selq]])
        cl = np.concatenate([col_ref[selr] - m * c.NLOC, q_c[selq] - m * c.NLOC])
        vv = np.concatenate([np.ones(int(selr.sum()), np.float32), q_v[selq]])
        sb_pc.append((bb // 128, bb % 128, cl, vv))
    sa_idx, sa_val, sa_off, sa_wmap, saT = _tile_stream(c, sa_pc, c.NWS, c.N)
    sb_idx, sb_val, sb_off, sb_wmap, sbT = _tile_stream(c, sb_pc, c.NWS, c.NLOC)

    # session_len layout for per-partition scale: lenr[p, i] = len[128*i + p]
    lenr = sl.reshape(c.BT, 128).T.astype(np.float32).copy()

    da_t = (D @ A).T.astype(BF16NP)                          # DA^T, bf16
    dab = np.concatenate([da_t[i * 128:(i + 1) * 128, :] for i in range(c.BT)],
                         axis=1)                             # [128, BT*B]
    # row-shard DA^T across cores (16 partition rows each), allgathered on
    # device: rank m's flat slab is exactly dab rows [m*16, (m+1)*16)
    da_sh = [np.ascontiguousarray(dab[m * 16:(m + 1) * 16, :]).reshape(
        128, c.BT * c.B // 8) for m in range(c.M)]
    wtp = np.zeros((128, 2 * c.EMB), BF16NP)
    for i in range(w_sess.shape[0]):
        wtp[:c.EMB, i * c.EMB:(i + 1) * c.EMB] = w_sess[i].T.astype(BF16NP)

    embc = c.NLOC * c.EMB // 128

    def b8(x):
        return np.ascontiguousarray(x).view(np.int8)

    lw = (7 * l1_cap + 3) // 4 * 4
    in_maps = []
    for m in range(c.M):
        emb_r = emb_q[m * c.NLOC:(m + 1) * c.NLOC].reshape(128, embc)
        # single byte-blob per core: [idx i32 | val bf16 | da | wt | off | emb]
        blob = np.concatenate([
            b8(np.concatenate([sa_idx[m], sb_idx[m]], axis=1)),
            b8(np.concatenate([sa_val[m], sb_val[m]], axis=1)),
            b8(da_sh[m]), b8(wtp),
            np.concatenate([sa_off[m], sb_off[m]], axis=1),
            emb_r,
        ], axis=1)
        pad = (-blob.shape[1]) % 4
        if pad:
            blob = np.concatenate(
                [blob, np.zeros((128, pad), np.int8)], axis=1)
        lp = np.concatenate([
            b8(l1_idx[m].reshape(128, c.G, l1_cap)).reshape(128, c.G, -1),
            b8(l1_val[m].reshape(128, c.G, l1_cap)).reshape(128, c.G, -1),
            l1_off[m].reshape(128, c.G, l1_cap),
            np.zeros((128, c.G, lw - 7 * l1_cap), np.int8),
        ], axis=2)
        in_maps.append({
            "blob": np.ascontiguousarray(blob),
            "l1p": np.ascontiguousarray(lp),
            "lenr": lenr,
        })

    plan = {"l1_cap": l1_cap, "saT": saT, "sbT": sbT,
            "sa_wmap": sa_wmap.tolist(), "sb_wmap": sb_wmap.tolist()}
    return plan, in_maps


# ---------------------------------------------------------------------------
# Bass program (identical on all cores; per-core behavior comes from inputs)
# ---------------------------------------------------------------------------

CHUNK_TILES = 32


def _chunks(total, size):
    out = []
    s = 0
    while s < total:
        out.append((s, min(size, total - s)))
        s += size
    return out


def build_program(cfg, plan):
    c = cfg
    nc = bacc.Bacc("TRN2", target_bir_lowering=False, debug=False,
                   num_devices=c.M)

    embc = c.NLOC * c.EMB // 128
    l1_cap, saT, sbT = plan["l1_cap"], plan["saT"], plan["sbT"]
    assert l1_cap <= CHUNK_TILES, f"l1_cap {l1_cap} exceeds gather buffers"
    dshc = c.BT * c.B // 8
    i_cols = saT + sbT
    # byte bases inside the blob: [idx i32 | val bf16 | da | wt | off | emb]
    idx0 = 0
    val0 = idx0 + i_cols * 4
    da0 = val0 + i_cols * 2
    wt0 = da0 + dshc * 2
    off0 = wt0 + 2 * c.EMB * 2
    emb0 = off0 + i_cols
    bw = (emb0 + embc + 3) // 4 * 4
    cb = {
        "sa_idx": idx0, "sb_idx": idx0 + saT * 4,
        "sa_val": val0, "sb_val": val0 + saT * 2,
        "sa_off": off0, "sb_off": off0 + saT,
        "da": da0, "wt": wt0, "emb": emb0,
    }
    lw = (7 * l1_cap + 3) // 4 * 4

    blob_t = nc.dram_tensor("blob", [128, bw], I8, kind="ExternalInput")
    l1p_t = nc.dram_tensor("l1p", [128, c.G, lw], I8, kind="ExternalInput")
    lenr_t = nc.dram_tensor("lenr", [128, c.BT], F32, kind="ExternalInput")
    out_t = nc.dram_tensor("out", [c.B, c.EMB], BF16, kind="ExternalOutput")

    emb_loc_t = nc.dram_tensor("emb_loc", [128, embc], I8,
                               kind="Internal")
    emb_full_t = nc.dram_tensor("emb_full", [c.NTOT, c.EMB], I8,
                                kind="Internal", addr_space="Shared")
    da_loc_t = nc.dram_tensor("da_loc", [128, c.BT * c.B // 8], BF16,
                              kind="Internal")
    da_full_t = nc.dram_tensor("da_full", [128, c.BT * c.B], BF16,
                               kind="Internal", addr_space="Shared")
    cur1_t = nc.dram_tensor("cur1", [c.G, 128, c.EMB], BF16, kind="Internal")
    ar_in_t = nc.dram_tensor("ar_in", [c.EMB, c.B], F32, kind="Internal")
    ar_out_t = nc.dram_tensor("ar_out", [c.EMB, c.B], F32, kind="Internal",
                              addr_space="Shared")

    with tile.TileContext(nc) as tc, ExitStack() as ctx:
        _body(ctx, tc, c, plan, cb, blob_t, l1p_t, emb_loc_t, emb_full_t,
              da_loc_t, da_full_t, lenr_t, out_t, cur1_t, ar_in_t, ar_out_t)

    nc.compile()
    return nc


def _body(ctx, tc, c, plan, cb, blob_t, l1p_t, emb_loc_t, emb_full_t,
          da_loc_t, da_full_t, lenr_t, out_t, cur1_t, ar_in_t, ar_out_t):
    nc = tc.nc
    CT = CHUNK_TILES
    embc = c.NLOC * c.EMB // 128

    const_p = ctx.enter_context(tc.tile_pool(name="const", bufs=1))
    ident = const_p.tile([128, 128], F32)
    make_identity(nc, ident[:])
    iota = const_p.tile([128, 128], I8)
    nc.gpsimd.iota(iota[:], pattern=[[1, 128]], base=0, channel_multiplier=0,
                   allow_small_or_imprecise_dtypes=True)

    # ------------- phase 0: assemble the full int8 table on device ---------
    # collectives cannot read IO tensors -> stage shard into Internal DRAM
    nc.sync.dma_start(emb_loc_t[:], blob_t[:, cb["emb"]:cb["emb"] + embc])
    nc.gpsimd.collective_compute(
        "AllGather", mybir.AluOpType.bypass,
        replica_groups=[list(range(c.M))],
        ins=[emb_loc_t.ap().opt()], outs=[emb_full_t.ap().opt()])
    dshc = c.BT * c.B // 8
    nc.sync.dma_start(da_loc_t[:],
                      blob_t[:, cb["da"]:cb["da"] + dshc * 2].bitcast(BF16))
    nc.gpsimd.collective_compute(
        "AllGather", mybir.AluOpType.bypass,
        replica_groups=[list(range(c.M))],
        ins=[da_loc_t.ap().opt()], outs=[da_full_t.ap().opt()])

    # persistent gather chunk buffers (manual double buffer, memset once so
    # OOB-skipped slots never expose NaN garbage to the matmul).  emb_full
    # gathers land in int8 buffers and are cast to bf16; cur1 gathers land
    # in the bf16 buffers directly.
    gb_p = ctx.enter_context(tc.tile_pool(name="gbuf", bufs=1))
    gbufs = [gb_p.tile([128, CT, c.EMB], BF16, tag=f"gb{i}", name=f"gb{i}")
             for i in range(2)]
    gb8s = [gb_p.tile([128, CT, c.EMB], I8, tag=f"g8{i}", name=f"g8{i}")
            for i in range(2)]
    for t in gbufs:
        nc.vector.memset(t[:], 0.0)

    sess_ps_p = ctx.enter_context(tc.tile_pool(name="sessps", bufs=1, space="PSUM"))
    sess_ps = sess_ps_p.tile([c.EMB, c.B], F32)

    gb_i = 0

    def load_chunk(mi_p, idx_base, off_base, val_base, ch_start, ch_n):
        s0 = idx_base + ch_start * 4
        idx_sb = mi_p.tile([128, CT], I32, tag="idx")
        nc.sync.dma_start(idx_sb[:, :ch_n],
                          blob_t[:, s0:s0 + ch_n * 4].bitcast(I32))
        s0 = off_base + ch_start
        off_sb = mi_p.tile([128, CT], I8, tag="off")
        nc.sync.dma_start(off_sb[:, :ch_n], blob_t[:, s0:s0 + ch_n])
        msk_sb = mi_p.tile([128, CT, 128], BF16, tag="msk")
        off_b = off_sb[:, :ch_n].unsqueeze(2).broadcast_to([128, ch_n, 128])
        iota_b = iota[:].unsqueeze(1).broadcast_to([128, ch_n, 128])
        nc.vector.tensor_tensor(msk_sb[:, :ch_n, :], off_b, iota_b,
                                mybir.AluOpType.is_equal)
        s0 = val_base + ch_start * 2
        val_sb = mi_p.tile([128, CT], BF16, tag="val")
        nc.sync.dma_start(val_sb[:, :ch_n],
                          blob_t[:, s0:s0 + ch_n * 2].bitcast(BF16))
        val_b = val_sb[:, :ch_n].unsqueeze(2).broadcast_to([128, ch_n, 128])
        nc.vector.tensor_tensor(msk_sb[:, :ch_n, :], msk_sb[:, :ch_n, :],
                                val_b, mybir.AluOpType.mult)
        return idx_sb, msk_sb

    def gather_chunk(gb, gb8, idx_sb, ch_n, src_t, nrows, axis=0):
        """Gather rows of src_t; int8 sources land in gb8 then cast to gb.
        axis picks the offset axis whose trailing extent is one row (EMB)."""
        tgt = gb8 if gb8 is not None else gb
        for tt in range(ch_n):
            nc.gpsimd.indirect_dma_start(
                out=tgt[:, tt, :], out_offset=None,
                in_=src_t[:],
                in_offset=bass.IndirectOffsetOnAxis(
                    ap=idx_sb[:, tt:tt + 1], axis=axis),
                bounds_check=nrows - 1, oob_is_err=False)
        if gb8 is not None:
            nc.vector.tensor_copy(gb[:, :ch_n, :], gb8[:, :ch_n, :])

    # ---------------- phase 1: cur1 = S @ emb (local row shard) ------------
    # One hardware-loop iteration per 128-row group: uniform l1_cap tiles,
    # so the PSUM start/stop flags are position-static inside the body.
    CAP = plan["l1_cap"]
    with tc.tile_pool(name="l1mi", bufs=1) as mi_p, \
         tc.tile_pool(name="l1ps", bufs=1, space="PSUM") as ps_p, \
         tc.tile_pool(name="l1tp", bufs=1, space="PSUM") as tp_p, \
         tc.tile_pool(name="l1st", bufs=1) as st_p:
        with tc.For_i(0, c.G) as g:
            idx_sb = mi_p.tile([128, CAP], I32, tag="idx")
            nc.sync.dma_start(idx_sb[:], l1p_t[:, g, 0:4 * CAP].bitcast(I32))
            off_sb = mi_p.tile([128, CAP], I8, tag="off")
            nc.sync.dma_start(off_sb[:], l1p_t[:, g, 6 * CAP:7 * CAP])
            val_sb = mi_p.tile([128, CAP], BF16, tag="val")
            nc.sync.dma_start(val_sb[:],
                              l1p_t[:, g, 4 * CAP:6 * CAP].bitcast(BF16))
            msk_sb = mi_p.tile([128, CAP, 128], BF16, tag="msk")
            off_b = off_sb[:].unsqueeze(2).broadcast_to([128, CAP, 128])
            iota_b = iota[:].unsqueeze(1).broadcast_to([128, CAP, 128])
            nc.vector.tensor_tensor(msk_sb[:], off_b, iota_b,
                                    mybir.AluOpType.is_equal)
            val_b = val_sb[:].unsqueeze(2).broadcast_to([128, CAP, 128])
            nc.vector.tensor_tensor(msk_sb[:], msk_sb[:], val_b,
                                    mybir.AluOpType.mult)
            gb, gb8 = gbufs[0], gb8s[0]
            for tt in range(CAP):
                nc.gpsimd.indirect_dma_start(
                    out=gb8[:, tt, :], out_offset=None,
                    in_=emb_full_t[:],
                    in_offset=bass.IndirectOffsetOnAxis(
                        ap=idx_sb[:, tt:tt + 1], axis=0),
                    bounds_check=c.N - 1, oob_is_err=False)
            nc.vector.tensor_copy(gb[:, :CAP, :], gb8[:, :CAP, :])
            ps = ps_p.tile([c.EMB, 128], F32, tag="ps")
            for tt in range(CAP):
                nc.tensor.matmul(
                    out=ps[:], lhsT=gb[:, tt, :], rhs=msk_sb[:, tt, :],
                    start=(tt == 0), stop=(tt == CAP - 1))
            s1 = st_p.tile([c.EMB, 128], F32, tag="s1")
            nc.vector.tensor_copy(s1[:], ps[:])
            ps2 = tp_p.tile([128, c.EMB], F32, tag="tp")
            nc.tensor.transpose(out=ps2[:], in_=s1[:],
                                identity=ident[:c.EMB, :c.EMB])
            s2 = st_p.tile([128, c.EMB], BF16, tag="s2")
            nc.vector.tensor_copy(s2[:], ps2[:])
            nc.sync.dma_start(cur1_t[g], s2[:])

    # ---------------- phase 2: pooled partial sums into sess_ps ------------
    streams = [
        ("sa_idx", "sa_val", "sa_off", plan["saT"], plan["sa_wmap"],
         emb_full_t, c.N, True, 0),
        ("sb_idx", "sb_val", "sb_off", plan["sbT"], plan["sb_wmap"],
         cur1_t, c.NLOC, False, 1),
    ]
    n_sess_mm = plan["saT"] + plan["sbT"]
    mm_i = 0
    with tc.tile_pool(name="smi", bufs=3) as mi_p:
        for idx_k, val_k, off_k, T, swmap, src_t, nrows, is_i8, gaxis in streams:
            for ch_start, ch_n in _chunks(T, CT):
                idx_sb, msk_sb = load_chunk(
                    mi_p, cb[idx_k], cb[off_k], cb[val_k], ch_start, ch_n)
                gb = gbufs[gb_i % 2]
                gb8 = gb8s[gb_i % 2] if is_i8 else None
                gb_i += 1
                gather_chunk(gb, gb8, idx_sb, ch_n, src_t, nrows, axis=gaxis)
                for tt in range(ch_n):
                    w = swmap[ch_start + tt]
                    nc.tensor.matmul(
                        out=sess_ps[:, w * 128:(w + 1) * 128],
                        lhsT=gb[:, tt, :],
                        rhs=msk_sb[:, tt, :],
                        start=(mm_i == 0), stop=(mm_i == n_sess_mm - 1))
                    mm_i += 1

    # ---------------- phase 3: AllReduce + dense tail ----------------------
    with tc.tile_pool(name="tail", bufs=1) as tp, \
         tc.tile_pool(name="tailps", bufs=1, space="PSUM") as tps, \
         tc.tile_pool(name="tailps2", bufs=1, space="PSUM") as tps2, \
         tc.tile_pool(name="tmp", bufs=2) as tmp_p:
        sess_sb = tp.tile([c.EMB, c.B], F32, tag="sess_sb")
        nc.vector.tensor_copy(sess_sb[:], sess_ps[:])
        nc.sync.dma_start(ar_in_t[:], sess_sb[:])
        nc.gpsimd.collective_compute(
            "AllReduce", mybir.AluOpType.add,
            replica_groups=[list(range(c.M))],
            ins=[ar_in_t.ap().opt()], outs=[ar_out_t.ap().opt()])
        sess_all = tp.tile([c.EMB, c.B], F32, tag="sess_all")
        nc.sync.dma_start(sess_all[:], ar_out_t[:])

        lr = tp.tile([128, c.BT], F32, tag="lr")
        nc.sync.dma_start(lr[:], lenr_t[:])
        rc3 = tp.tile([128, c.BT], F32, tag="rc3")
        nc.vector.reciprocal(rc3[:], lr[:])
        nc.vector.tensor_scalar_mul(rc3[:], rc3[:], 1.0 / 3.0)

        # sess0 b-tiles (scaled) + acc + back-transpose to e-layout
        accs = [tp.tile([128, c.EMB], F32, tag=f"acc{i}", name=f"acc{i}")
                for i in range(c.BT)]
        cur_e = tp.tile([c.EMB, c.B], BF16, tag="cur_e0")
        for i in range(c.BT):
            pst = tps.tile([128, c.EMB], F32, tag="tp")
            nc.tensor.transpose(out=pst[:],
                                in_=sess_all[:, i * 128:(i + 1) * 128],
                                identity=ident[:c.EMB, :c.EMB])
            s0 = tmp_p.tile([128, c.EMB], F32, tag="s0")
            nc.scalar.mul(s0[:], pst[:], rc3[:, i:i + 1])
            nc.vector.tensor_copy(accs[i][:], s0[:])
            pse = tps2.tile([c.EMB, 128], F32, tag="tpe")
            nc.tensor.transpose(out=pse[:], in_=s0[:], identity=ident[:, :])
            nc.vector.tensor_copy(cur_e[:, i * 128:(i + 1) * 128], pse[:])

        # DA^T b'-tiles (host-precomputed, sharded upload + allgather)
        da_sb = [tp.tile([128, c.B], BF16, tag=f"da{i}", name=f"dasb{i}")
                 for i in range(c.BT)]
        for i in range(c.BT):
            nc.sync.dma_start(da_sb[i][:], da_full_t[:, i * c.B:(i + 1) * c.B])

        wt_sb = tp.tile([c.EMB, 2, c.EMB], BF16, tag="wt")
        for i in range(2):
            s0 = cb["wt"] + i * c.EMB * 2
            nc.sync.dma_start(
                wt_sb[:, i, :],
                blob_t[:c.EMB, s0:s0 + c.EMB * 2].bitcast(BF16))

        for layer in range(2):
            psy = tps.tile([c.EMB, c.B], F32, tag="ypsum")
            nc.tensor.matmul(out=psy[:], lhsT=wt_sb[:, layer, :], rhs=cur_e[:],
                             start=True, stop=True)
            y_e = tmp_p.tile([c.EMB, c.B], F32, tag="y_e")
            nc.vector.tensor_copy(y_e[:], psy[:])
            y_b = []
            for bt in range(c.BT):
                pst = tps.tile([128, c.EMB], F32, tag="tp")
                nc.tensor.transpose(out=pst[:],
                                    in_=y_e[:, bt * 128:(bt + 1) * 128],
                                    identity=ident[:c.EMB, :c.EMB])
                yb = tmp_p.tile([128, c.EMB], BF16, tag=f"yb{bt}")
                nc.vector.tensor_copy(yb[:], pst[:])
                y_b.append(yb)
            if layer == 0:
                cur_e = tp.tile([c.EMB, c.B], BF16, tag="cur_e1")
            for bt in range(c.BT):
                psz = tps.tile([128, c.EMB], F32, tag="zps")
                for k in range(c.BT):
                    nc.tensor.matmul(out=psz[:],
                                     lhsT=da_sb[k][:, bt * 128:(bt + 1) * 128],
                                     rhs=y_b[k][:],
                                     start=(k == 0), stop=(k == c.BT - 1))
                z = tmp_p.tile([128, c.EMB], F32, tag=f"z{bt}")
                nc.vector.tensor_copy(z[:], psz[:])
                sq = tmp_p.tile([128, c.EMB], F32, tag="sq")
                nc.vector.tensor_mul(sq[:], z[:], z[:])
                ss = tmp_p.tile([128, 1], F32, tag="ss")
                nc.vector.tensor_reduce(ss[:], sq[:], mybir.AxisListType.X,
                                        mybir.AluOpType.add)
                nrm = tmp_p.tile([128, 1], F32, tag="nrm")
                nc.scalar.sqrt(nrm[:], ss[:])
                nc.vector.tensor_scalar_max(nrm[:], nrm[:], 1e-12)
                rn = tmp_p.tile([128, 1], F32, tag="rn")
                nc.vector.reciprocal(rn[:], nrm[:])
                zn = tmp_p.tile([128, c.EMB], F32, tag=f"zn{bt}")
                nc.scalar.mul(zn[:], z[:], rn[:])
                nc.vector.tensor_add(accs[bt][:], accs[bt][:], zn[:])
                if layer == 0:
                    pse = tps2.tile([c.EMB, 128], F32, tag="tpe")
                    nc.tensor.transpose(out=pse[:], in_=zn[:],
                                        identity=ident[:, :])
                    nc.vector.tensor_copy(cur_e[:, bt * 128:(bt + 1) * 128],
                                          pse[:])

        for bt in range(c.BT):
            ot = tmp_p.tile([128, c.EMB], BF16, tag="ot")
            nc.scalar.mul(ot[:], accs[bt][:], 1.0 / 3.0)
            nc.sync.dma_start(out_t[bt * 128:(bt + 1) * 128, :], ot[:])


# ---------------------------------------------------------------------------

def run_on_hw(cfg, plan, nc, in_maps):
    from concourse.bass_utils import run_bass_kernel_spmd
    res = run_bass_kernel_spmd(nc, in_maps, core_ids=list(range(cfg.M)))
    return res


def kernel(**inputs):
    cfg = Cfg()
    plan, in_maps = prep(cfg, inputs)
    nc = build_program(cfg, plan)
    res = run_on_hw(cfg, plan, nc, in_maps)
    out = np.asarray(res.results[0]["out"]).astype(np.float32)
    return out


# revision 66
# speedup vs baseline: 1.3405x; 1.3405x over previous
"""Trainium2 Bass kernel for COTREC-style GNN message passing.

Math (reference):
    cur1 = S @ emb                      (S sparse [N,N], 1M nnz)
    cur2 = S @ cur1
    item = (emb + cur1 + cur2) / 3
    sess = meanpool_sessions(item)      ([B, E])
    ... small dense tail (DA @ ..., w_sess, l2norm) ...

Device decomposition (8 cores, SPMD single program, per-core data via inputs):
  * The embedding table is uploaded bf16, row-sharded (one shard per core),
    and AllGathered on device into a full DRAM copy per core - 8x less
    host->device traffic than replicating it.
  * Row-shard nodes: core m owns rows [m*NLOC, (m+1)*NLOC).
  * cur1 shard computed locally as a stream of 128-slot tiles: each tile
    gathers up to 128 edge-source rows (indirect DMA, OOB pad slots
    skipped via bounds_check) and one mask-matmul [128 slots x 128 rows]
    segment-sums them into the 128-row group's PSUM accumulator. Masks are
    built ON DEVICE from per-slot (val bf16, row-offset u8) via a
    broadcast is_equal against an iota - only 7B/slot goes over the wire.
  * cur2 never materialized. The session pooling is pushed through the
    graph: sess*3*len = P01@emb + P01@cur1 + (P01@S)@cur1, where P01/Q01
    structures are host-computed integer index work, processed as the same
    kind of tile stream. Per-core partial pooled sums [E, B] -> one small
    AllReduce.
  * Dense tail replicated on all cores (DA = D @ A precomputed on host,
    shipped bf16); core 0's output returned.
  * All per-core arrays ship in 4 consolidated [128, x] blobs (bf16 /
    int32 / uint8 / f32) to minimise per-buffer dispatch overhead.
"""

import os
import sys
from contextlib import ExitStack

import numpy as np
import ml_dtypes

BF16NP = ml_dtypes.bfloat16

for _p in ("/opt/trn_rl_repo", os.path.expanduser("~/.axon_site/_ro/trn_rl_repo")):
    if os.path.isdir(_p) and _p not in sys.path:
        sys.path.append(_p)

import concourse.bacc as bacc
import concourse.bass as bass
import concourse.tile as tile
from concourse import mybir
from concourse.masks import make_identity

F32 = mybir.dt.float32
BF16 = mybir.dt.bfloat16
I32 = mybir.dt.int32
I8 = mybir.dt.int8


class Cfg:
    def __init__(self, N=100000, NNZ=1000000, B=512, L=50, M=8, EMB=112):
        self.N, self.NNZ, self.B, self.L, self.M = N, NNZ, B, L, M
        self.EMB = EMB
        # shard rows per core: multiple of 128, cover N
        self.NLOC = ((N + M - 1) // M + 127) // 128 * 128
        self.NTOT = self.NLOC * M          # rows in allgathered table
        self.G = self.NLOC // 128          # 128-row groups per core
        self.NWS = B // 128                # session windows (128 wide)
        self.BT = B // 128                 # tail b-tiles


# ---------------------------------------------------------------------------
# Host preprocessing: pure integer/layout work + permutation of input floats.
# ---------------------------------------------------------------------------

def _csr_expand(rowptr, rows):
    """For each r in rows return concatenated [rowptr[r], rowptr[r+1]) ranges."""
    deg = rowptr[rows + 1] - rowptr[rows]
    total = int(deg.sum())
    if total == 0:
        return np.zeros(0, np.int64), deg
    cum = np.cumsum(deg)
    out = np.arange(total, dtype=np.int64) - np.repeat(cum - deg, deg) \
        + np.repeat(rowptr[rows], deg)
    return out, deg


def _tile_stream(cfg, per_core, n_win, oob, uniform=False):
    """Pack per-core (window, col-offset, gather-idx, val) edge lists into a
    shared-layout stream of 128-slot tiles.

    per_core[m] = (win, off, gidx, val) arrays.  Tile capacity per window is
    the max over cores (the SPMD program must be identical on every core).
    With uniform=True every window gets the same capacity (hardware-loop
    friendly: tile t belongs to window t // cap).
    Returns (idx[M,128,T], val[M,128,T], off[M,128,T], wmap[T], T).
    """
    c = cfg
    cnts = np.zeros((c.M, n_win), np.int64)
    for m, (w, _, _, _) in enumerate(per_core):
        cnts[m] = np.bincount(w, minlength=n_win)
    caps = np.maximum(1, (cnts.max(axis=0) + 127) // 128)    # tiles per window
    if uniform:
        caps[:] = caps.max()
    tbase = np.zeros(n_win, np.int64)
    tbase[1:] = np.cumsum(caps)[:-1]
    T = int(caps.sum())
    wmap = np.zeros(T, np.int64)
    for wi in range(n_win):
        wmap[tbase[wi]:tbase[wi] + caps[wi]] = wi
    idx = np.full((c.M, 128, T), oob, np.int32)
    val = np.zeros((c.M, 128, T), BF16NP)
    off = np.full((c.M, 128, T), -1, np.int8)            # -1 = empty slot
    for m, (w, o, gidx, vv) in enumerate(per_core):
        so = np.argsort(w, kind="stable")
        w, o, gidx, vv = w[so], o[so], gidx[so], vv[so]
        starts = np.zeros(n_win, np.int64)
        starts[1:] = np.cumsum(cnts[m])[:-1]
        k = np.arange(len(w)) - starts[w]               # rank within window
        t = tbase[w] + k // 128
        p = k % 128
        idx[m, p, t] = gidx
        val[m, p, t] = vv.astype(BF16NP)
        off[m, p, t] = o
    return idx, val, off, wmap, T


def prep(cfg, inputs):
    """Build per-core input arrays + the (core-independent) program plan."""
    c = cfg
    emb = np.asarray(inputs["embedding"], np.float32)
    av = np.asarray(inputs["adj_vals"], np.float32)
    ar = np.asarray(inputs["adj_rows"], np.int64)
    ac = np.asarray(inputs["adj_cols"], np.int64)
    D = np.asarray(inputs["D"], np.float32)
    A = np.asarray(inputs["A"], np.float32)
    si = np.asarray(inputs["session_item"], np.int64)
    sl = np.asarray(inputs["session_len"], np.float32)
    w_sess = np.asarray(inputs["w_sess"], np.float32)

    # int8 per-row-quantized sharded embedding table (zero rows pad to NTOT);
    # the per-row scale is folded into the edge-mask values (bf16) below.
    s_row = np.maximum(np.abs(emb).max(axis=1) / 127.0, 1e-12)  # [N]
    emb_q = np.zeros((c.NTOT, c.EMB), np.int8)
    emb_q[:c.N] = np.clip(np.round(emb / s_row[:, None]), -127, 127)

    # session refs: (b, col) for non-pad items
    b_ref = np.repeat(np.arange(c.B, dtype=np.int64), c.L)
    it_ref = si.ravel()
    keep = it_ref > 0
    b_ref, col_ref = b_ref[keep], it_ref[keep] - 1       # cols in [0, N)

    # CSR of S by row (for Q01 = P01 @ S)
    order = np.argsort(ar, kind="stable")
    ar_s, ac_s, av_s = ar[order], ac[order], av[order]
    rowptr = np.searchsorted(ar_s, np.arange(c.N + 1)).astype(np.int64)
    epos, deg = _csr_expand(rowptr, col_ref)
    q_b = np.repeat(b_ref, deg)
    q_c = ac_s[epos]
    q_v = av_s[epos]

    # prune L1 edges to rows actually referenced by T1/T2
    ref_mask = np.zeros(c.N, bool)
    ref_mask[col_ref] = True
    ref_mask[q_c] = True
    ekeep = ref_mask[ar]
    er, ec, ev = ar[ekeep], ac[ekeep], av[ekeep]

    own_e = er // c.NLOC
    own_ref = col_ref // c.NLOC
    own_q = q_c // c.NLOC

    # ---- L1 stream: windows are the 128-row groups of the local shard ----
    l1_pc = []
    for m in range(c.M):
        sel = own_e == m
        r = er[sel] - m * c.NLOC
        l1_pc.append((r // 128, r % 128, ec[sel], ev[sel] * s_row[ec[sel]]))
    l1_idx, l1_val, l1_off, _, l1T = _tile_stream(c, l1_pc, c.G, c.N,
                                                  uniform=True)
    l1_cap = l1T // c.G

    # ---- sess stream A: P01 @ emb (gathers emb at col_ref) ----------------
    # ---- sess stream B: (P01 + Q01) @ cur1 (gathers local cur1 rows) ------
    sa_pc = []
    sb_pc = []
    for m in range(c.M):
        selr = own_ref == m
        sa_pc.append((b_ref[selr] // 128, b_ref[selr] % 128, col_ref[selr],
                      s_row[col_ref[selr]]))
        selq = own_q == m
        bb = np.concatenate([b_ref[selr], q_b[selq]])
        cl = np.concatenate([col_ref[selr] - m * c.NLOC, q_c[selq] - m * c.NLOC])
        vv = np.concatenate([np.ones(int(selr.sum()), np.float32), q_v[selq]])
        sb_pc.append((bb // 128, bb % 128, cl, vv))
    sa_idx, sa_val, sa_off, sa_wmap, saT = _tile_stream(c, sa_pc, c.NWS, c.N)
    sb_idx, sb_val, sb_off, sb_wmap, sbT = _tile_stream(c, sb_pc, c.NWS, c.NLOC)

    # session_len layout for per-partition scale: lenr[p, i] = len[128*i + p]
    lenr = sl.reshape(c.BT, 128).T.astype(np.float32).copy()

    da_t = (D @ A).T.astype(BF16NP)                          # DA^T, bf16
    dab = np.concatenate([da_t[i * 128:(i + 1) * 128, :] for i in range(c.BT)],
                         axis=1)                             # [128, BT*B]
    # row-shard DA^T across cores (16 partition rows each), allgathered on
    # device: rank m's flat slab is exactly dab rows [m*16, (m+1)*16)
    da_sh = [np.ascontiguousarray(dab[m * 16:(m + 1) * 16, :]).reshape(
        128, c.BT * c.B // 8) for m in range(c.M)]
    wtp = np.zeros((128, 2 * c.EMB), BF16NP)
    for i in range(w_sess.shape[0]):
        wtp[:c.EMB, i * c.EMB:(i + 1) * c.EMB] = w_sess[i].T.astype(BF16NP)

    embc = c.NLOC * c.EMB // 128

    in_maps = []
    for m in range(c.M):
        emb_r = emb_q[m * c.NLOC:(m + 1) * c.NLOC].reshape(128, embc)
        bfb = np.concatenate(
            [sa_val[m], sb_val[m], da_sh[m], wtp], axis=1)
        i32b = np.concatenate([sa_idx[m], sb_idx[m]], axis=1)
        i8b = np.concatenate([emb_r, sa_off[m], sb_off[m]], axis=1)
        in_maps.append({
            "bfb": np.ascontiguousarray(bfb),
            "i32b": np.ascontiguousarray(i32b),
            "i8b": np.ascontiguousarray(i8b),
            "l1i": np.ascontiguousarray(l1_idx[m].reshape(128, c.G, l1_cap)),
            "l1v": np.ascontiguousarray(l1_val[m].reshape(128, c.G, l1_cap)),
            "l1o": np.ascontiguousarray(l1_off[m].reshape(128, c.G, l1_cap)),
            "lenr": lenr,
        })

    plan = {"l1_cap": l1_cap, "saT": saT, "sbT": sbT,
            "sa_wmap": sa_wmap.tolist(), "sb_wmap": sb_wmap.tolist()}
    return plan, in_maps


# ---------------------------------------------------------------------------
# Bass program (identical on all cores; per-core behavior comes from inputs)
# ---------------------------------------------------------------------------

CHUNK_TILES = 32


def _chunks(total, size):
    out = []
    s = 0
    while s < total:
        out.append((s, min(size, total - s)))
        s += size
    return out


def build_program(cfg, plan):
    c = cfg
    nc = bacc.Bacc("TRN2", target_bir_lowering=False, debug=False,
                   num_devices=c.M)

    embc = c.NLOC * c.EMB // 128
    l1_cap, saT, sbT = plan["l1_cap"], plan["saT"], plan["sbT"]
    assert l1_cap <= CHUNK_TILES, f"l1_cap {l1_cap} exceeds gather buffers"
    # blob column bases
    dshc = c.BT * c.B // 8
    cb = {
        "sa_val": 0, "sb_val": saT,
        "da": saT + sbT, "wt": saT + sbT + dshc,
        "sa_idx": 0, "sb_idx": saT,
        "emb": 0, "sa_off": embc, "sb_off": embc + saT,
    }
    bf_cols = cb["wt"] + 2 * c.EMB
    i_cols = saT + sbT

    bfb_t = nc.dram_tensor("bfb", [128, bf_cols], BF16, kind="ExternalInput")
    i32b_t = nc.dram_tensor("i32b", [128, i_cols], I32, kind="ExternalInput")
    i8b_t = nc.dram_tensor("i8b", [128, embc + i_cols], I8, kind="ExternalInput")
    l1i_t = nc.dram_tensor("l1i", [128, c.G, l1_cap], I32, kind="ExternalInput")
    l1v_t = nc.dram_tensor("l1v", [128, c.G, l1_cap], BF16, kind="ExternalInput")
    l1o_t = nc.dram_tensor("l1o", [128, c.G, l1_cap], I8, kind="ExternalInput")
    lenr_t = nc.dram_tensor("lenr", [128, c.BT], F32, kind="ExternalInput")
    out_t = nc.dram_tensor("out", [c.B, c.EMB], BF16, kind="ExternalOutput")

    emb_loc_t = nc.dram_tensor("emb_loc", [128, embc], I8,
                               kind="Internal")
    emb_full_t = nc.dram_tensor("emb_full", [c.NTOT, c.EMB], I8,
                                kind="Internal", addr_space="Shared")
    da_loc_t = nc.dram_tensor("da_loc", [128, c.BT * c.B // 8], BF16,
                              kind="Internal")
    da_full_t = nc.dram_tensor("da_full", [128, c.BT * c.B], BF16,
                               kind="Internal", addr_space="Shared")
    cur1_t = nc.dram_tensor("cur1", [c.G, 128, c.EMB], BF16, kind="Internal")
    ar_in_t = nc.dram_tensor("ar_in", [c.EMB, c.B], F32, kind="Internal")
    ar_out_t = nc.dram_tensor("ar_out", [c.EMB, c.B], F32, kind="Internal",
                              addr_space="Shared")

    with tile.TileContext(nc) as tc, ExitStack() as ctx:
        _body(ctx, tc, c, plan, cb, bfb_t, i32b_t, i8b_t, l1i_t, l1v_t,
              l1o_t, emb_loc_t, emb_full_t, da_loc_t, da_full_t, lenr_t,
              out_t, cur1_t, ar_in_t, ar_out_t)

    nc.compile()
    return nc


def _body(ctx, tc, c, plan, cb, bfb_t, i32b_t, i8b_t, l1i_t, l1v_t,
          l1o_t, emb_loc_t, emb_full_t, da_loc_t, da_full_t, lenr_t,
          out_t, cur1_t, ar_in_t, ar_out_t):
    nc = tc.nc
    CT = CHUNK_TILES
    embc = c.NLOC * c.EMB // 128

    const_p = ctx.enter_context(tc.tile_pool(name="const", bufs=1))
    ident = const_p.tile([128, 128], F32)
    make_identity(nc, ident[:])
    iota = const_p.tile([128, 128], I8)
    nc.gpsimd.iota(iota[:], pattern=[[1, 128]], base=0, channel_multiplier=0,
                   allow_small_or_imprecise_dtypes=True)

    # ------------- phase 0: assemble the full int8 table on device ---------
    # collectives cannot read IO tensors -> stage shard into Internal DRAM
    nc.sync.dma_start(emb_loc_t[:], i8b_t[:, cb["emb"]:cb["emb"] + embc])
    nc.gpsimd.collective_compute(
        "AllGather", mybir.AluOpType.bypass,
        replica_groups=[list(range(c.M))],
        ins=[emb_loc_t.ap().opt()], outs=[emb_full_t.ap().opt()])
    dshc = c.BT * c.B // 8
    nc.sync.dma_start(da_loc_t[:], bfb_t[:, cb["da"]:cb["da"] + dshc])
    nc.gpsimd.collective_compute(
        "AllGather", mybir.AluOpType.bypass,
        replica_groups=[list(range(c.M))],
        ins=[da_loc_t.ap().opt()], outs=[da_full_t.ap().opt()])

    # persistent gather chunk buffers (manual double buffer, memset once so
    # OOB-skipped slots never expose NaN garbage to the matmul).  emb_full
    # gathers land in int8 buffers and are cast to bf16; cur1 gathers land
    # in the bf16 buffers directly.
    gb_p = ctx.enter_context(tc.tile_pool(name="gbuf", bufs=1))
    gbufs = [gb_p.tile([128, CT, c.EMB], BF16, tag=f"gb{i}", name=f"gb{i}")
             for i in range(2)]
    gb8s = [gb_p.tile([128, CT, c.EMB], I8, tag=f"g8{i}", name=f"g8{i}")
            for i in range(2)]
    for t in gbufs:
        nc.vector.memset(t[:], 0.0)

    sess_ps_p = ctx.enter_context(tc.tile_pool(name="sessps", bufs=1, space="PSUM"))
    sess_ps = sess_ps_p.tile([c.EMB, c.B], F32)

    gb_i = 0

    def load_chunk(mi_p, idx_base, off_base, val_base, ch_start, ch_n):
        s0 = idx_base + ch_start
        idx_sb = mi_p.tile([128, CT], I32, tag="idx")
        nc.sync.dma_start(idx_sb[:, :ch_n], i32b_t[:, s0:s0 + ch_n])
        s0 = off_base + ch_start
        off_sb = mi_p.tile([128, CT], I8, tag="off")
        nc.sync.dma_start(off_sb[:, :ch_n], i8b_t[:, s0:s0 + ch_n])
        msk_sb = mi_p.tile([128, CT, 128], BF16, tag="msk")
        off_b = off_sb[:, :ch_n].unsqueeze(2).broadcast_to([128, ch_n, 128])
        iota_b = iota[:].unsqueeze(1).broadcast_to([128, ch_n, 128])
        nc.vector.tensor_tensor(msk_sb[:, :ch_n, :], off_b, iota_b,
                                mybir.AluOpType.is_equal)
        s0 = val_base + ch_start
        val_sb = mi_p.tile([128, CT], BF16, tag="val")
        nc.sync.dma_start(val_sb[:, :ch_n], bfb_t[:, s0:s0 + ch_n])
        val_b = val_sb[:, :ch_n].unsqueeze(2).broadcast_to([128, ch_n, 128])
        nc.vector.tensor_tensor(msk_sb[:, :ch_n, :], msk_sb[:, :ch_n, :],
                                val_b, mybir.AluOpType.mult)
        return idx_sb, msk_sb

    def gather_chunk(gb, gb8, idx_sb, ch_n, src_t, nrows, axis=0):
        """Gather rows of src_t; int8 sources land in gb8 then cast to gb.
        axis picks the offset axis whose trailing extent is one row (EMB)."""
        tgt = gb8 if gb8 is not None else gb
        for tt in range(ch_n):
            nc.gpsimd.indirect_dma_start(
                out=tgt[:, tt, :], out_offset=None,
                in_=src_t[:],
                in_offset=bass.IndirectOffsetOnAxis(
                    ap=idx_sb[:, tt:tt + 1], axis=axis),
                bounds_check=nrows - 1, oob_is_err=False)
        if gb8 is not None:
            nc.vector.tensor_copy(gb[:, :ch_n, :], gb8[:, :ch_n, :])

    # ---------------- phase 1: cur1 = S @ emb (local row shard) ------------
    # One hardware-loop iteration per 128-row group: uniform l1_cap tiles,
    # so the PSUM start/stop flags are position-static inside the body.
    CAP = plan["l1_cap"]
    with tc.tile_pool(name="l1mi", bufs=1) as mi_p, \
         tc.tile_pool(name="l1ps", bufs=1, space="PSUM") as ps_p, \
         tc.tile_pool(name="l1tp", bufs=1, space="PSUM") as tp_p, \
         tc.tile_pool(name="l1st", bufs=1) as st_p:
        with tc.For_i(0, c.G) as g:
            idx_sb = mi_p.tile([128, CAP], I32, tag="idx")
            nc.sync.dma_start(idx_sb[:], l1i_t[:, g, :])
            off_sb = mi_p.tile([128, CAP], I8, tag="off")
            nc.sync.dma_start(off_sb[:], l1o_t[:, g, :])
            val_sb = mi_p.tile([128, CAP], BF16, tag="val")
            nc.sync.dma_start(val_sb[:], l1v_t[:, g, :])
            msk_sb = mi_p.tile([128, CAP, 128], BF16, tag="msk")
            off_b = off_sb[:].unsqueeze(2).broadcast_to([128, CAP, 128])
            iota_b = iota[:].unsqueeze(1).broadcast_to([128, CAP, 128])
            nc.vector.tensor_tensor(msk_sb[:], off_b, iota_b,
                                    mybir.AluOpType.is_equal)
            val_b = val_sb[:].unsqueeze(2).broadcast_to([128, CAP, 128])
            nc.vector.tensor_tensor(msk_sb[:], msk_sb[:], val_b,
                                    mybir.AluOpType.mult)
            gb, gb8 = gbufs[0], gb8s[0]
            for tt in range(CAP):
                nc.gpsimd.indirect_dma_start(
                    out=gb8[:, tt, :], out_offset=None,
                    in_=emb_full_t[:],
                    in_offset=bass.IndirectOffsetOnAxis(
                        ap=idx_sb[:, tt:tt + 1], axis=0),
                    bounds_check=c.N - 1, oob_is_err=False)
            nc.vector.tensor_copy(gb[:, :CAP, :], gb8[:, :CAP, :])
            ps = ps_p.tile([c.EMB, 128], F32, tag="ps")
            for tt in range(CAP):
                nc.tensor.matmul(
                    out=ps[:], lhsT=gb[:, tt, :], rhs=msk_sb[:, tt, :],
                    start=(tt == 0), stop=(tt == CAP - 1))
            s1 = st_p.tile([c.EMB, 128], F32, tag="s1")
            nc.vector.tensor_copy(s1[:], ps[:])
            ps2 = tp_p.tile([128, c.EMB], F32, tag="tp")
            nc.tensor.transpose(out=ps2[:], in_=s1[:],
                                identity=ident[:c.EMB, :c.EMB])
            s2 = st_p.tile([128, c.EMB], BF16, tag="s2")
            nc.vector.tensor_copy(s2[:], ps2[:])
            nc.sync.dma_start(cur1_t[g], s2[:])

    # ---------------- phase 2: pooled partial sums into sess_ps ------------
    streams = [
        ("sa_idx", "sa_val", "sa_off", plan["saT"], plan["sa_wmap"],
         emb_full_t, c.N, True, 0),
        ("sb_idx", "sb_val", "sb_off", plan["sbT"], plan["sb_wmap"],
         cur1_t, c.NLOC, False, 1),
    ]
    n_sess_mm = plan["saT"] + plan["sbT"]
    mm_i = 0
    with tc.tile_pool(name="smi", bufs=3) as mi_p:
        for idx_k, val_k, off_k, T, swmap, src_t, nrows, is_i8, gaxis in streams:
            for ch_start, ch_n in _chunks(T, CT):
                idx_sb, msk_sb = load_chunk(
                    mi_p, cb[idx_k], cb[off_k], cb[val_k], ch_start, ch_n)
                gb = gbufs[gb_i % 2]
                gb8 = gb8s[gb_i % 2] if is_i8 else None
                gb_i += 1
                gather_chunk(gb, gb8, idx_sb, ch_n, src_t, nrows, axis=gaxis)
                for tt in range(ch_n):
                    w = swmap[ch_start + tt]
                    nc.tensor.matmul(
                        out=sess_ps[:, w * 128:(w + 1) * 128],
                        lhsT=gb[:, tt, :],
                        rhs=msk_sb[:, tt, :],
                        start=(mm_i == 0), stop=(mm_i == n_sess_mm - 1))
                    mm_i += 1

    # ---------------- phase 3: AllReduce + dense tail ----------------------
    with tc.tile_pool(name="tail", bufs=1) as tp, \
         tc.tile_pool(name="tailps", bufs=1, space="PSUM") as tps, \
         tc.tile_pool(name="tailps2", bufs=1, space="PSUM") as tps2, \
         tc.tile_pool(name="tmp", bufs=2) as tmp_p:
        sess_sb = tp.tile([c.EMB, c.B], F32, tag="sess_sb")
        nc.vector.tensor_copy(sess_sb[:], sess_ps[:])
        nc.sync.dma_start(ar_in_t[:], sess_sb[:])
        nc.gpsimd.collective_compute(
            "AllReduce", mybir.AluOpType.add,
            replica_groups=[list(range(c.M))],
            ins=[ar_in_t.ap().opt()], outs=[ar_out_t.ap().opt()])
        sess_all = tp.tile([c.EMB, c.B], F32, tag="sess_all")
        nc.sync.dma_start(sess_all[:], ar_out_t[:])

        lr = tp.tile([128, c.BT], F32, tag="lr")
        nc.sync.dma_start(lr[:], lenr_t[:])
        rc3 = tp.tile([128, c.BT], F32, tag="rc3")
        nc.vector.reciprocal(rc3[:], lr[:])
        nc.vector.tensor_scalar_mul(rc3[:], rc3[:], 1.0 / 3.0)

        # sess0 b-tiles (scaled) + acc + back-transpose to e-layout
        accs = [tp.tile([128, c.EMB], F32, tag=f"acc{i}", name=f"acc{i}")
                for i in range(c.BT)]
        cur_e = tp.tile([c.EMB, c.B], BF16, tag="cur_e0")
        for i in range(c.BT):
            pst = tps.tile([128, c.EMB], F32, tag="tp")
            nc.tensor.transpose(out=pst[:],
                                in_=sess_all[:, i * 128:(i + 1) * 128],
                                identity=ident[:c.EMB, :c.EMB])
            s0 = tmp_p.tile([128, c.EMB], F32, tag="s0")
            nc.scalar.mul(s0[:], pst[:], rc3[:, i:i + 1])
            nc.vector.tensor_copy(accs[i][:], s0[:])
            pse = tps2.tile([c.EMB, 128], F32, tag="tpe")
            nc.tensor.transpose(out=pse[:], in_=s0[:], identity=ident[:, :])
            nc.vector.tensor_copy(cur_e[:, i * 128:(i + 1) * 128], pse[:])

        # DA^T b'-tiles (host-precomputed, sharded upload + allgather)
        da_sb = [tp.tile([128, c.B], BF16, tag=f"da{i}", name=f"dasb{i}")
                 for i in range(c.BT)]
        for i in range(c.BT):
            nc.sync.dma_start(da_sb[i][:], da_full_t[:, i * c.B:(i + 1) * c.B])

        wt_sb = tp.tile([c.EMB, 2, c.EMB], BF16, tag="wt")
        for i in range(2):
            s0 = cb["wt"] + i * c.EMB
            nc.sync.dma_start(wt_sb[:, i, :], bfb_t[:c.EMB, s0:s0 + c.EMB])

        for layer in range(2):
            psy = tps.tile([c.EMB, c.B], F32, tag="ypsum")
            nc.tensor.matmul(out=psy[:], lhsT=wt_sb[:, layer, :], rhs=cur_e[:],
                             start=True, stop=True)
            y_e = tmp_p.tile([c.EMB, c.B], F32, tag="y_e")
            nc.vector.tensor_copy(y_e[:], psy[:])
            y_b = []
            for bt in range(c.BT):
                pst = tps.tile([128, c.EMB], F32, tag="tp")
                nc.tensor.transpose(out=pst[:],
                                    in_=y_e[:, bt * 128:(bt + 1) * 128],
                                    identity=ident[:c.EMB, :c.EMB])
                yb = tmp_p.tile([128, c.EMB], BF16, tag=f"yb{bt}")
                nc.vector.tensor_copy(yb[:], pst[:])
                y_b.append(yb)
            if layer == 0:
                cur_e = tp.tile([c.EMB, c.B], BF16, tag="cur_e1")
            for bt in range(c.BT):
                psz = tps.tile([128, c.EMB], F32, tag="zps")
                for k in range(c.BT):
                    nc.tensor.matmul(out=psz[:],
                                     lhsT=da_sb[k][:, bt * 128:(bt + 1) * 128],
                                     rhs=y_b[k][:],
                                     start=(k == 0), stop=(k == c.BT - 1))
                z = tmp_p.tile([128, c.EMB], F32, tag=f"z{bt}")
                nc.vector.tensor_copy(z[:], psz[:])
                sq = tmp_p.tile([128, c.EMB], F32, tag="sq")
                nc.vector.tensor_mul(sq[:], z[:], z[:])
                ss = tmp_p.tile([128, 1], F32, tag="ss")
                nc.vector.tensor_reduce(ss[:], sq[:], mybir.AxisListType.X,
                                        mybir.AluOpType.add)
                nrm = tmp_p.tile([128, 1], F32, tag="nrm")
                nc.scalar.sqrt(nrm[:], ss[:])
                nc.vector.tensor_scalar_max(nrm[:], nrm[:], 1e-12)
                rn = tmp_p.tile([128, 1], F32, tag="rn")
                nc.vector.reciprocal(rn[:], nrm[:])
                zn = tmp_p.tile([128, c.EMB], F32, tag=f"zn{bt}")
                nc.scalar.mul(zn[:], z[:], rn[:])
                nc.vector.tensor_add(accs[bt][:], accs[bt][:], zn[:])
                if layer == 0:
                    pse = tps2.tile([c.EMB, 128], F32, tag="tpe")
                    nc.tensor.transpose(out=pse[:], in_=zn[:],
                                        identity=ident[:, :])
                    nc.vector.tensor_copy(cur_e[:, bt * 128:(bt + 1) * 128],
                                          pse[:])

        for bt in range(c.BT):
            ot = tmp_p.tile([128, c.EMB], BF16, tag="ot")
            nc.scalar.mul(ot[:], accs[bt][:], 1.0 / 3.0)
            nc.sync.dma_start(out_t[bt * 128:(bt + 1) * 128, :], ot[:])


# ---------------------------------------------------------------------------

def run_on_hw(cfg, plan, nc, in_maps):
    from concourse.bass_utils import run_bass_kernel_spmd
    res = run_bass_kernel_spmd(nc, in_maps, core_ids=list(range(cfg.M)))
    return res


def kernel(**inputs):
    cfg = Cfg()
    plan, in_maps = prep(cfg, inputs)
    nc = build_program(cfg, plan)
    res = run_on_hw(cfg, plan, nc, in_maps)
    out = np.asarray(res.results[0]["out"]).astype(np.float32)
    return out
